# revision 3
# baseline (speedup 1.0000x reference)
"""Trainium2 Bass kernel for the autoregressive TCN decoder (nn_Decoder).

Strategy: wavefront Gauss-Seidel fixed-point decode.
----------------------------------------------------
The reference decodes T=32 positions sequentially (each step re-runs the
full TCN).  Instead we iterate the sequence-level fixed point: start from
y=0 feedback, and repeatedly recompute the TCN over position chunks using
the latest available feedback (Gauss-Seidel over chunks, Jacobi within a
chunk).  The iteration contracts at ~0.35x per pass; NPASS passes bring
the output error orders of magnitude below the 2e-2 gate.

Work is organized as (pass, chunk) tiles on a 2D grid.  chunk c of pass p
depends only on (p, c-1) [left context + fresh y for its first position]
and (p-1, c) [its own previous values]; both lie on the previous
antidiagonal, so tiles are emitted antidiagonal-by-antidiagonal and the
Tile framework pipelines independent tiles across engines (pass p+1 chunk
c runs concurrently with pass p chunk c+1).

Per-core layout: batch-sharded (8 rows/core), channels on partitions,
columns = position*8+batch with a 16-position zero pad on the left so
causal taps never need masking.  Per-layer activation tiles are
double-buffered by pass parity to avoid write-after-read hazards between
concurrent wavefront tiles.  Conv weights/activations are fp16 (fp32 PSUM
accumulation), conv_in runs in fp32 (x tile holds the fp32 y feedback).
ELU sites: ScalarE computes e=exp(z+b) (table-exact); a custom 4-stage DVE
op finishes elu(z+b) = min(e, max(z+b+1, 1)) - 1 in one instruction.
Residual sites avoid the ACT engine entirely with a 2-op DVE polynomial
exp: elu(s) via p=P3(max(s/32,-1)), e=p^32 (rel err ~6e-4 after the 5
squarings, far under tolerance).
"""
import os
import sys
import functools
import numpy as np

# recover automatically if a previous run left the NeuronCores wedged
os.environ.setdefault("NEURON_RT_RESET_CORES", "1")

sys.path.insert(0, "/opt/trn_rl_repo")

import concourse.bass as bass
import concourse.bacc as bacc
import concourse.mybir as mybir
from concourse.tile import TileContext
from concourse.bass_utils import run_bass_kernel_spmd

F32 = mybir.dt.float32
F16 = mybir.dt.float16
AF = mybir.ActivationFunctionType
OP = mybir.AluOpType

# architecture constants
T = 32
B = 64
NCORES = 8
BC = B // NCORES          # 8 batch rows per core
F = 128
CIN = 33
NB = 8
DILS = [1, 2, 4, 8, 1, 2, 4, 8]

# schedule constants
WCH = 16                  # chunk width in positions
NCH = T // WCH
NPASS = 8                 # GS passes (HW rel err 5.5e-3 vs 2e-2 gate)
PADP = 16                 # zero-pad positions on the left
NPOS = PADP + T + 1       # one extra slot so feedback writes never clip
NCOL = NPOS * BC          # 392 columns
PADC = PADP * BC          # 128 pad columns
WC = WCH * BC             # 128 columns per chunk

# poly-exp coefficients: with u = min(-s/16, 1), exp(s) ~= P(u)^16 where
# P(u) = 1 + A1 u + A2 u^2 + A3 u^3 (Remez fit of exp(-u) on [0,1], a0=1)
A1 = -0.993630083
A2 = 0.463556795
A3 = -0.102190816
PSCALE = -1.0 / 16.0

LAST_EXEC_NS = None

_OPS = {}


def _register_ops():
    """Register the custom DVE ops (idempotent; rows 17-19 are free)."""
    if _OPS:
        return _OPS
    from concourse import dve_ops as DO
    from concourse.dve_spec import (
        Spec, Src0, Src1, C0, C1, C2, Zero, One, maxx, minn, sq, lower,
        _has_src1,
    )
    from concourse.dve_uop import DveOpSpec

    def mk(name, row, spec):
        if name in DO._SUB_OPCODE_FOR_NAME:
            op = next(o for o in DO.OPS if o.name == name)
            return op
        shas = {}
        for ver in ("v3", "v4"):
            s = DveOpSpec(name=name, opcode=row, uops=lower(spec, ver=ver),
                          rd1_en=_has_src1(spec))
            shas[ver] = s.sha(ver)
        op = DO.DveOp(name, spec, subdim=False, uops_sha=shas)
        DO.OPS.append(op)
        DO._SUB_OPCODE_FOR_NAME[name] = row
        DO.CUSTOM_DVE_SPECS[name] = spec
        return op

    # out = min(e, max(z + c0, 1)) - 1      (elu from exact exp; c0 = bias+1)
    elu_tail = mk("ELU_TAIL_ANT", 17, Spec(
        body=minn(Src1, maxx(Src0 + C0, One)) - One,
        reference=lambda in0, in1, c0, c1, c2: np.asarray(
            np.minimum(in1, np.maximum(in0 + c0, 1.0)) - 1.0, np.float32),
    ))

    # u = min(in0*imm2, 1); out = ((in1*u + c0)*u + c1)*u + 1
    # (in1 = a3 column, c0 = a2, c1 = a1, imm2 = -1/16)
    exp_poly = mk("EXP_POLY_ANT", 18, Spec(
        body=(lambda u: ((Src1 * u + C0) * u + C1) * u + One)(
            minn(Src0 * C2, One)),
        reference=lambda in0, in1, c0, c1, c2: np.asarray(
            (lambda u: ((in1 * u + c0) * u + c1) * u + 1.0)(
                np.minimum(in0 * c2, 1.0)), np.float32),
    ))

    # out = min(in1^16, max(in0 + c0, 1)) - 1   (c0 = bias+1)
    sq4_tail = mk("SQ4_TAIL_ANT", 19, Spec(
        body=minn(sq(sq(sq(sq(Src1)))), maxx(Src0 + C0, One)) - One,
        reference=lambda in0, in1, c0, c1, c2: np.asarray(
            np.minimum((in1.astype(np.float64) ** 16).astype(np.float32),
                       np.maximum(in0 + c0, 1.0)) - 1.0, np.float32),
    ))

    _OPS.update(elu_tail=elu_tail, exp_poly=exp_poly, sq4_tail=sq4_tail)
    return _OPS


def build_nc(reps=1, npass=NPASS, res_act=False, interleave=True, dbg="",
             wch=WCH, split_w=True):
    ops = _register_ops()
    elu_tail, exp_poly, sq4_tail = ops["elu_tail"], ops["exp_poly"], ops["sq4_tail"]
    WC = wch * BC
    NCH = T // wch

    nc = bacc.Bacc("TRN2", target_bir_lowering=False, debug=False)

    xdec_d = nc.declare_dram_parameter("xdec", [32, T * BC], F32, isOutput=False)
    ylast_d = nc.declare_dram_parameter("ylast", [1, BC], F32, isOutput=False)
    win_d = nc.declare_dram_parameter("win", [CIN, 3 * F], F32, isOutput=False)
    wres_d = nc.declare_dram_parameter("wres", [CIN, F], F32, isOutput=False)
    w1_d = nc.declare_dram_parameter("w1l", [F, 7 * 3 * F], F16, isOutput=False)
    w2_d = nc.declare_dram_parameter("w2l", [F, 8 * 3 * F], F16, isOutput=False)
    wd0t_d = nc.declare_dram_parameter("wd0t", [F, F], F16, isOutput=False)
    wd1_d = nc.declare_dram_parameter("wd1", [F, 64], F16, isOutput=False)
    wd2_d = nc.declare_dram_parameter("wd2", [64, 1], F16, isOutput=False)
    encT_d = nc.declare_dram_parameter("encT", [BC, F], F16, isOutput=False)
    sel_d = nc.declare_dram_parameter("sel", [BC, T * BC], F16, isOutput=False)
    bias_d = nc.declare_dram_parameter("biasC", [F, 40], F32, isOutput=False)
    out_d = nc.declare_dram_parameter("out", [1, T * BC], F32, isOutput=True)

    # bias column map: conv l in [0,16): col 2l = b_l, col 2l+1 = b_l + 1
    # col 32 = bd0, 33 = bd0+1, 34 = bd1, 35 = bd1+1, 36 = a3, 37 = bd2
    with TileContext(nc) as tc:
        with (
            tc.tile_pool(name="consts", bufs=1) as cpool,
            tc.tile_pool(name="steps", bufs=4) as spool,
            tc.tile_pool(name="ps", bufs=1, space="PSUM") as ppool,
        ):
            win_sb = cpool.tile([CIN, 3 * F], F32, name="win_sb")
            wres_sb = cpool.tile([CIN, F], F32, name="wres_sb")
            w1_sb = cpool.tile([F, 7 * 3 * F], F16, name="w1_sb")
            w2_sb = cpool.tile([F, 8 * 3 * F], F16, name="w2_sb")
            wd0t_sb = cpool.tile([F, F], F16, name="wd0t_sb")
            wd1_sb = cpool.tile([F, 64], F16, name="wd1_sb")
            wd2_sb = cpool.tile([64, 1], F16, name="wd2_sb")
            encT_sb = cpool.tile([BC, F], F16, name="encT_sb")
            sel_sb = cpool.tile([BC, T * BC], F16, name="sel_sb")
            bias_sb = cpool.tile([F, 40], F32, name="bias_sb")
            dummy = cpool.tile([1, 1], F32, name="dummy")

            # activation tiles, double-buffered by pass parity
            xt = [cpool.tile([CIN, NCOL], F32, name=f"xt{par}") for par in range(2)]
            ctl = [[cpool.tile([F, NCOL], F16, name=f"ct{b}_{par}")
                    for par in range(2)] for b in range(NB)]
            htl = [[cpool.tile([F, NCOL], F16, name=f"ht{b}_{par}")
                    for par in range(2)] for b in range(NB)]
            rest = [cpool.tile([F, NCOL], F16, name=f"res{par}") for par in range(2)]

            # ---- prologue: DMAs (chunked to 128-col pieces) ----
            for sb, dr in [
                (win_sb, win_d), (wres_sb, wres_d),
                (w1_sb, w1_d), (w2_sb, w2_d),
                (wd0t_sb, wd0t_d), (wd1_sb, wd1_d), (wd2_sb, wd2_d),
                (encT_sb, encT_d), (sel_sb, sel_d), (bias_sb, bias_d),
            ]:
                ncols = sb.shape[-1]
                for c0 in range(0, ncols, F):
                    c1 = min(c0 + F, ncols)
                    nc.sync.dma_start(sb[:, c0:c1], dr[:, c0:c1])

            # zero + fill the activation tiles
            for par in range(2):
                nc.vector.memset(xt[par][:, :], 0.0)
                nc.gpsimd.memset(rest[par][:, :], 0.0)
                for b in range(NB):
                    nc.vector.memset(ctl[b][par][:, :], 0.0)
                    nc.gpsimd.memset(htl[b][par][:, :], 0.0)
            for par in range(2):
                nc.sync.dma_start(xt[par][0:32, PADC:PADC + T * BC], xdec_d[:, :])
                nc.sync.dma_start(xt[par][32:33, PADC:PADC + BC], ylast_d[:, :])

            # warm the Exp table during DMA wait
            nc.scalar.activation(dummy[:, :], bias_sb[0:1, 0:1], AF.Exp)

            # PE warm-up touches: one tiny matmul per DMA'd weight chunk so
            # later matmuls carry at most one sync wait each
            pswu = ppool.tile([1, 1], F32, name="pswu", tag="psy", bufs=1)
            wu_sb = cpool.tile([F, 1], F32, name="wu_sb")
            for sb in (win_sb, wres_sb, w1_sb, w2_sb, wd0t_sb, wd1_sb,
                       wd2_sb, encT_sb, sel_sb, bias_sb):
                p = sb.shape[0]
                for c0 in range(0, sb.shape[-1], F):
                    ap = sb[0:p, c0:c0 + 1]
                    nc.tensor.matmul(pswu[:, :], ap, ap, start=True, stop=True)
                    nc.vector.tensor_copy(wu_sb[0:p, :], ap)
            for par in range(2):
                for ap in (xt[par][:, 0:1], xt[par][:, PADC:PADC + 1],
                           xt[par][:, 256:257]):
                    nc.tensor.matmul(pswu[:, :], ap, ap, start=True, stop=True)
                    nc.vector.tensor_copy(wu_sb[0:CIN, :], ap)

            # a3 as a full tile: [P,1]-broadcast Src1 wedges the DVE on HW
            a3w = cpool.tile([F, WC], F32, name="a3w")
            nc.vector.memset(a3w[:, :], A3)

            def bcol(i):
                return bias_sb[:, i:i + 1]

            # ---- site helpers ----
            def site_act(ps, bc0, bc1, out_ap, p=F, wc=WC):
                """out = elu(z + b) with e on ScalarE + one DVE tail op."""
                if "nocustom" in dbg:
                    nc.vector.tensor_scalar(out_ap, ps[0:p, 0:wc],
                                            bc0[0:p, :], None, op0=OP.add)
                    return
                if "noact" in dbg:
                    t_t = spool.tile([F, WC], F32, name="t_t", tag="e_t")
                    nc.vector.tensor_scalar(t_t[0:p, 0:wc], ps[0:p, 0:wc],
                                            bc0[0:p, :], None, op0=OP.add)
                    nc.vector._custom_dve(
                        elu_tail, out=out_ap, in0=ps[0:p, 0:wc],
                        in1=t_t[0:p, 0:wc], s0=bc1[0:p, :])
                    return
                e_t = spool.tile([F, WC], F32, name="e_t", tag="e_t")
                nc.scalar.activation(e_t[0:p, 0:wc], ps[0:p, 0:wc], AF.Exp,
                                     bias=bc0[0:p, :])
                nc.vector._custom_dve(
                    elu_tail, out=out_ap, in0=ps[0:p, 0:wc],
                    in1=e_t[0:p, 0:wc], s0=bc1[0:p, :])

            def site_res(c_ap, r_ap, out_ap):
                """out = elu(c + r): DVE-only (poly exp) or ACT-form."""
                s_t = spool.tile([F, WC], F32, name="s_t", tag="s_t")
                nc.vector.tensor_tensor(s_t[:, :], c_ap, r_ap, op=OP.add)
                if "nocustom" in dbg:
                    nc.vector.tensor_scalar(out_ap, s_t[:, :], 0.0, None,
                                            op0=OP.add)
                    return
                if res_act:
                    e_t = spool.tile([F, WC], F32, name="e_t", tag="e_t")
                    nc.scalar.activation(e_t[:, :], s_t[:, :], AF.Exp)
                    nc.vector._custom_dve(
                        elu_tail, out=out_ap, in0=s_t[:, :], in1=e_t[:, :],
                        s0=1.0)
                    return
                p_t = spool.tile([F, WC], F32, name="p_t", tag="p_t")
                nc.vector._custom_dve(
                    exp_poly, out=p_t[:, :], in0=s_t[:, :], in1=a3w[:, :],
                    s0=A2, s1=A1, imm2=PSCALE)
                nc.vector._custom_dve(
                    sq4_tail, out=out_ap, in0=s_t[:, :], in1=p_t[:, :], s0=1.0)

            def wsl(wt, base, k):
                return wt[:, (base + k) * F:(base + k + 1) * F]

            def conv3(ps, wt, base, tin, nin, d, lo, hi, extra_first=None):
                if extra_first is not None:
                    extra_first()
                for k in range(3):
                    sh = (2 - k) * d * BC
                    W = wsl(wt, base, k)
                    rhs = tin[0:nin, lo - sh:hi - sh]
                    if split_w and nin == F:
                        # 4x 32-col weight pieces load on separate XBUSes,
                        # cutting the serial LDWEIGHTS cost ~4x (HW only;
                        # CoreSim rejects per-partition-range psum groups)
                        for j in range(4):
                            nc.tensor.matmul(
                                ps[32 * j:32 * j + 32, :],
                                W[:, 32 * j:32 * j + 32], rhs,
                                start=(k == 0 and extra_first is None),
                                stop=(k == 2),
                                tile_position=(0, 32 * j),
                                skip_group_check=True)
                        continue
                    nc.tensor.matmul(
                        ps[:, :], W, rhs,
                        start=(k == 0 and extra_first is None), stop=(k == 2))

            # ---- main wavefront ----
            # chunk(p, c) is a generator yielding after each pipeline leg so
            # the driver can interleave emission across concurrent chains
            # (in-order engine streams would otherwise head-of-line block).
            def chunk(p, c):
                par, npar = p % 2, (p + 1) % 2
                lo, hi = PADC + c * WC, PADC + (c + 1) * WC
                x = xt[par]

                # block 0: conv_in (fp32) + res projection
                ps = ppool.tile([F, WC], F32, name="psA", tag="psA", bufs=3)
                conv3(ps, win_sb, 0, x, CIN, 1, lo, hi)
                psR = ppool.tile([F, WC], F32, name="psR", tag="psB", bufs=3)
                nc.tensor.matmul(psR[:, :], wres_sb[:, :], x[0:CIN, lo:hi],
                                 start=True, stop=True)
                yield
                site_act(ps, bcol(0), bcol(1), ctl[0][par][:, lo:hi])
                nc.vector.tensor_scalar(rest[par][:, lo:hi], psR[:, :],
                                        bcol(38)[0:F, :], None, op0=OP.add)
                yield
                ps = ppool.tile([F, WC], F32, name="psB", tag="psB", bufs=3)
                conv3(ps, w2_sb, 0, ctl[0][par], F, 1, lo, hi)
                yield
                c2_t = spool.tile([F, WC], F16, name="c2_t", tag="c2_t")
                site_act(ps, bcol(2), bcol(3), c2_t[:, :])
                site_res(c2_t[:, :], rest[par][:, lo:hi], htl[0][par][:, lo:hi])
                yield

                for b in range(1, NB):
                    d = DILS[b]
                    l1 = 2 * b
                    ps = ppool.tile([F, WC], F32, name="psA", tag="psA", bufs=3)
                    conv3(ps, w1_sb, (b - 1) * 3, htl[b - 1][par], F, d, lo, hi)
                    yield
                    site_act(ps, bcol(2 * l1), bcol(2 * l1 + 1),
                             ctl[b][par][:, lo:hi])
                    yield
                    ps = ppool.tile([F, WC], F32, name="psB", tag="psB", bufs=3)
                    conv3(ps, w2_sb, b * 3, ctl[b][par], F, d, lo, hi)
                    yield
                    c2_t = spool.tile([F, WC], F16, name="c2_t", tag="c2_t")
                    site_act(ps, bcol(2 * l1 + 2), bcol(2 * l1 + 3), c2_t[:, :])
                    site_res(c2_t[:, :], htl[b - 1][par][:, lo:hi],
                             htl[b][par][:, lo:hi])
                    yield

                # head
                ps0 = ppool.tile([F, WC], F32, name="ps0", tag="psA", bufs=3)
                nc.tensor.matmul(ps0[:, :], encT_sb[:, :], sel_sb[:, 0:WC],
                                 start=True, stop=False)
                nc.tensor.matmul(ps0[:, :], wd0t_sb[:, :],
                                 htl[NB - 1][par][:, lo:hi],
                                 start=False, stop=True)
                yield
                o0_t = spool.tile([F, WC], F16, name="o0_t", tag="o0_t")
                site_act(ps0, bcol(32), bcol(33), o0_t[:, :])
                yield
                ps1 = ppool.tile([64, WC], F32, name="ps1", tag="ps1", bufs=1)
                nc.tensor.matmul(ps1[:, :], wd1_sb[:, :], o0_t[:, :],
                                 start=True, stop=True)
                yield
                o1_t = spool.tile([64, WC], F16, name="o1_t", tag="o1_t")
                site_act(ps1, bcol(34), bcol(35), o1_t[:, :], p=64)
                yield
                psy = ppool.tile([1, WC], F32, name="psy", tag="psy", bufs=1)
                nc.tensor.matmul(psy[:, :], wd2_sb[:, :], o1_t[:, :],
                                 start=True, stop=True)
                yield
                # y feedback: entry (pos+1) on both parities; bd2 via bias col
                nc.vector.tensor_scalar(
                    xt[par][32:33, lo + BC:hi + BC], psy[:, :],
                    bcol(37)[0:1, :], None, op0=OP.add)
                nc.vector.tensor_scalar(
                    xt[npar][32:33, lo + BC:hi + BC], psy[:, :],
                    bcol(37)[0:1, :], None, op0=OP.add)

            if "prologue" in dbg:
                npass = 0
            MAXA = 3 if wch >= 16 else 4
            for _ in range(reps):
                if not interleave:
                    for t in range(npass + NCH - 1):
                        for c in range(NCH):
                            p = t - c
                            if 0 <= p < npass:
                                for _leg in chunk(p, c):
                                    pass
                    continue
                # sliding-window wavefront: activate (p, c) once (p-1, c)
                # and (p, c-1) have finished emitting; round-robin active
                # chains leg by leg so engine streams stay interleaved
                done = {(-1, c) for c in range(NCH)} | \
                       {(p, -1) for p in range(npass)}
                pending = [(t - c, c)
                           for t in range(npass + NCH - 1)
                           for c in range(NCH) if 0 <= t - c < npass]
                active = []
                while pending or active:
                    while pending and len(active) < MAXA:
                        p, c = pending[0]
                        if (p - 1, c) in done and (p, c - 1) in done:
                            active.append(((p, c), chunk(p, c)))
                            pending.pop(0)
                        else:
                            break
                    for item in list(active):
                        key, g = item
                        if next(g, StopIteration) is StopIteration:
                            active.remove(item)
                            done.add(key)

            final_par = (npass - 1) % 2
            nc.sync.dma_start(out_d[:, :],
                              xt[final_par][32:33, PADC + BC:PADC + (T + 1) * BC])

    nc.compile()
    return nc


def prepare_in_maps(inputs):
    f32 = np.float32
    bf = np.float16

    def a(x):
        return np.ascontiguousarray(np.asarray(x, f32))

    dec = a(inputs["data_decoder"])          # [B,T,32]
    ly = a(inputs["last_y"])                 # [B]
    enc = a(inputs["data_encoder"])          # [B,128,128]
    w_in = a(inputs["w_in"])                 # [3,33,128]
    b_in = a(inputs["b_in"])
    w_res = a(inputs["w_res"])               # [1,33,128]
    b_res = a(inputs["b_res"])
    w1 = a(inputs["w1"])                     # [7,3,128,128]
    b1 = a(inputs["b1"])
    w2 = a(inputs["w2"])                     # [8,3,128,128]
    b2 = a(inputs["b2"])
    wd0 = a(inputs["wd0"])                   # [256,128]
    bd0 = a(inputs["bd0"])
    wd1 = a(inputs["wd1"])                   # [128,64]
    bd1 = a(inputs["bd1"])
    wd2 = a(inputs["wd2"])                   # [64,1]
    bd2 = a(inputs["bd2"])

    win_l = np.ascontiguousarray(w_in.transpose(1, 0, 2).reshape(CIN, 3 * F))
    w1_l = np.ascontiguousarray(
        w1.transpose(2, 0, 1, 3).reshape(F, 7 * 3 * F).astype(bf))
    w2_l = np.ascontiguousarray(
        w2.transpose(2, 0, 1, 3).reshape(F, 8 * 3 * F).astype(bf))

    # bias tile: conv l: col 2l = b_l, col 2l+1 = b_l + 1 (order: conv_in,
    # conv2_0, then (conv1_b, conv2_b) for b=1..7); head cols 32..35;
    # col 36 = A3; col 37 = bd2; col 38 = b_res
    biasC = np.zeros((F, 40), f32)
    convs = [b_in, b2[0]]
    for b in range(1, NB):
        convs += [b1[b - 1], b2[b]]
    for l, bb in enumerate(convs):
        biasC[:, 2 * l] = bb
        biasC[:, 2 * l + 1] = bb + 1.0
    biasC[:, 32] = bd0
    biasC[:, 33] = bd0 + 1.0
    biasC[:64, 34] = bd1
    biasC[:64, 35] = bd1 + 1.0
    biasC[:, 36] = A3
    biasC[:, 37] = bd2[0]
    biasC[:, 38] = b_res

    sel = np.ascontiguousarray(
        np.tile(np.eye(BC, dtype=f32), (1, T)).astype(bf))

    in_maps = []
    for c in range(NCORES):
        sl = slice(c * BC, (c + 1) * BC)
        xdec = np.ascontiguousarray(dec[sl].transpose(2, 1, 0).reshape(32, T * BC))
        ylast = np.ascontiguousarray(ly[sl].reshape(1, BC))
        encT = np.ascontiguousarray(
            (enc[sl, -1, :] @ wd0[F:]).astype(bf))        # [BC, 128]
        in_maps.append({
            "xdec": xdec, "ylast": ylast,
            "win": win_l, "wres": np.ascontiguousarray(w_res[0]),
            "w1l": w1_l, "w2l": w2_l,
            "wd0t": np.ascontiguousarray(wd0[:F].astype(bf)),
            "wd1": wd1.astype(bf), "wd2": wd2.astype(bf),
            "encT": encT, "sel": sel, "biasC": biasC,
        })
    return in_maps


@functools.lru_cache(maxsize=1)
def _built_nc():
    return build_nc()


def kernel(**inputs) -> np.ndarray:
    global LAST_EXEC_NS
    nc = _built_nc()
    in_maps = prepare_in_maps(inputs)
    trace = bool(os.environ.get("KERNEL_TRACE"))
    try:
        r = run_bass_kernel_spmd(nc, in_maps, list(range(NCORES)), trace=trace)
    except ModuleNotFoundError:
        r = run_bass_kernel_spmd(nc, in_maps, list(range(NCORES)), trace=False)
    LAST_EXEC_NS = r.exec_time_ns if r.exec_time_ns else r.mean_exec_time_ns
    outs = []
    for c in range(NCORES):
        o = np.asarray(r.results[c]["out"]).reshape(T, BC).T   # [BC, T]
        outs.append(o)
    return np.ascontiguousarray(np.concatenate(outs, axis=0).astype(np.float32))
